# revision 8
# baseline (speedup 1.0000x reference)
"""Inverse 3D Haar wavelet transform (stride-2 kernel-2 conv_transpose) on 8 trn2 cores.

coeffs: [4, 64, 17, 128, 128] f32, channel dim = 8 subbands x 8 channels.
out:    [4, 8, 33, 256, 256] f32,
  out[b,c,2t+i-1, 2h+j, 2w+k] = 0.3536 * sum_s (-1)^(i*s2 + j*s1 + k*s0) x[b,s,c,t,h,w]
  (frame t'=-1 dropped).

Sharding: pure data parallel over the 8 channels c (one per core); each core
sees its [4, 8, 17, 128, 128] slice and emits [4, 33, 256, 256].

This kernel runs fp16 end-to-end on device (graded tolerance is 2e-2; fp16
butterflies land ~4e-4), halving HBM traffic vs f32: 17.8 MB in + 17.8 MB out
per core ~= 98 us at the 360 GB/s DMA roofline.

Host side: pre-scale by 0.3536, cast fp16, transpose to [bt=68, h=128, (s,w)]
so loads are 2 KB-contiguous per (bt, h). Output is [136, 256, 256] with frame
f = 2*bt + i; each b's first frame (t=0, i=0, the dropped t'=-1) lands on
f = 34*b which the host discards - keeps every device store uniform.

Device per 4-bt chunk (17 chunks, partition dim = h = 128):
  - one 1 MB load DMA (SP queue)
  - DVE: stage1 (s2->i) 2 ops, stage2 (s1->j) 2 ops, stage3 k=0 add - all
    packed fp16 = 2x DVE mode
  - GPSIMD: stage3 k=1 sub
  - ACT: interleave copy (t,i,j,k,w) -> (t,i,j,w'=2w+k)
  - one 1 MB store DMA (ACT queue)
"""

import sys

sys.path.insert(0, "/opt/trn_rl_repo")

import numpy as np

import concourse.bass as bass
import concourse.bacc as bacc
import concourse.mybir as mybir
from concourse.tile import TileContext
from concourse import bass_utils

B, S, C, T_FULL, H, W = 4, 8, 8, 17, 128, 128
SCALE = 0.3536
NBT = B * T_FULL  # 68 flattened (b, t) slices
BT_CHUNK = 4      # bt slices per inner iteration (68 = 4 * 17, no runts)

_cache = {}


def _build():
    f16 = mybir.dt.float16
    nc = bacc.Bacc()
    x = nc.dram_tensor("x", [NBT, H, S * W], f16, kind="ExternalInput")
    y = nc.dram_tensor("y", [2 * NBT, 2 * H, 2 * W], f16, kind="ExternalOutput")

    # small chunks at the ends shorten pipeline fill and drain
    chunks = [1, 1, 2] + [4] * 15 + [2, 1, 1]
    assert sum(chunks) == NBT
    # frames 34*b (b's t'=-1, i.e. i=0 of bt = 17*b) are dropped by the host;
    # skip them in the stores
    garbage_f = {2 * 17 * b for b in range(B)}
    with TileContext(nc) as tc:
        with tc.tile_pool(name="xin", bufs=4) as xp, \
             tc.tile_pool(name="u", bufs=3) as up, \
             tc.tile_pool(name="v", bufs=3) as vp, \
             tc.tile_pool(name="o", bufs=3) as op_, \
             tc.tile_pool(name="f", bufs=4) as fp:
            bt0 = 0
            for cidx, TC in enumerate(chunks):
                tail = cidx >= len(chunks) - 3
                FD = TC * S * W
                # ---- load: one DMA, 2 KB descriptors per (bt, h)
                X = xp.tile([H, FD], f16, tag="x")
                nc.sync.dma_start(
                    out=X[:].rearrange("p (t f) -> p t f", t=TC),
                    in_=x[bt0:bt0 + TC].rearrange("t p f -> p t f"))
                # ---- stage 1 (DVE): u[i] = x[s2=0] +/- x[s2=1]
                U = up.tile([H, FD], f16, tag="u")
                X3 = X[:].rearrange("p (t h) -> p t h", t=TC)          # h=(s,w) 1024
                U3 = U[:].rearrange("p (t i h) -> p t i h", t=TC, i=2)  # h=(s1 s0 w) 512
                nc.vector.tensor_add(U3[:, :, 0], X3[:, :, :512], X3[:, :, 512:])
                nc.vector.tensor_sub(U3[:, :, 1], X3[:, :, :512], X3[:, :, 512:])
                # ---- stage 2 (DVE): v[i,j] = u[i,s1=0] +/- u[i,s1=1]
                V = vp.tile([H, FD], f16, tag="v")
                U4 = U[:].rearrange("p (t i s1 g) -> p t i s1 g", t=TC, i=2, s1=2)
                V4 = V[:].rearrange("p (t i j g) -> p t i j g", t=TC, i=2, j=2)
                nc.vector.tensor_add(V4[:, :, :, 0], U4[:, :, :, 0], U4[:, :, :, 1])
                nc.vector.tensor_sub(V4[:, :, :, 1], U4[:, :, :, 0], U4[:, :, :, 1])
                # ---- stage 3: o[i,j,k] = v[i,j,s0=0] +/- v[i,j,s0=1]
                #      k=1 on GPSIMD; k=0 split DVE/GPSIMD to balance the two
                #      (packed writes keep DVE in its 2x fp16 mode)
                O = op_.tile([H, FD], f16, tag="o")
                V5 = V[:].rearrange("p (t ij s0 w) -> p t ij s0 w", t=TC, ij=4, s0=2)
                O5 = O[:].rearrange("p (t ij k w) -> p t ij k w", t=TC, ij=4, k=2)
                nc.gpsimd.tensor_sub(O5[:, :, :, 1], V5[:, :, :, 0], V5[:, :, :, 1])
                tsp = TC - 1 if TC == 4 else TC  # last t of big chunks -> GPSIMD
                nc.vector.tensor_add(O5[:, :tsp, :, 0],
                                     V5[:, :tsp, :, 0], V5[:, :tsp, :, 1])
                if tsp < TC:
                    nc.gpsimd.tensor_add(O5[:, tsp:, :, 0],
                                         V5[:, tsp:, :, 0], V5[:, tsp:, :, 1])
                # ---- interleave copy (ACT): (t,i,j,k,w) -> (t,i,j, w'=2w+k)
                #      (per-frame on the tail chunks so stores can drain early)
                F = fp.tile([H, FD], f16, tag="f")
                Fv = F[:].rearrange("p (g j w k) -> p g j k w", g=2 * TC, j=2, k=2)
                Og = O[:].rearrange("p (g j k w) -> p g j k w", g=2 * TC, j=2, k=2)
                for g0, g1 in ([(0, TC), (TC, 2 * TC)] if tail
                               else [(0, 2 * TC)]):
                    nc.scalar.copy(out=Fv[:, g0:g1], in_=Og[:, g0:g1])
                # ---- store frame ranges (skipping dropped frames);
                #      F free = (g, j, w') with g = 2*t_local + i
                f0 = 2 * bt0
                Fg = F[:].rearrange("p (g j w) -> p g j w", g=2 * TC, j=2)
                cuts = sorted(f - f0 for f in garbage_f if f0 <= f < f0 + 2 * TC)
                if tail:
                    pairs = [(g, g + 1) for g in range(2 * TC)
                             if (f0 + g) not in garbage_f]  # per-frame stores
                else:
                    bounds = [0] + [c for cut in cuts
                                    for c in (cut, cut + 1)] + [2 * TC]
                    pairs = list(zip(bounds[::2], bounds[1::2]))
                for ga, gb in pairs:
                    if ga == gb:
                        continue
                    dst = y[f0 + ga:f0 + gb].rearrange("g (p j) w -> p g j w", j=2)
                    nc.scalar.dma_start(out=dst, in_=Fg[:, ga:gb])
                bt0 += TC
    nc.finalize()
    return nc


def kernel(coeffs: np.ndarray) -> np.ndarray:
    coeffs = np.asarray(coeffs, dtype=np.float32)
    if "nc" not in _cache:
        _cache["nc"] = _build()
    nc = _cache["nc"]
    in_maps = []
    for c in range(8):
        xc = coeffs[:, c::8]  # [b, s, t, h, w] subband-major channel slice
        xt = (xc * np.float32(SCALE)).astype(np.float16).transpose(0, 2, 3, 1, 4)
        in_maps.append({"x": np.ascontiguousarray(xt).reshape(NBT, H, S * W)})
    res = bass_utils.run_bass_kernel_spmd(nc, in_maps, core_ids=list(range(8)))
    # y frame f = 2*bt + i = 34*b + 2*t + i; frame 34*b is the dropped t'=-1
    out = np.stack(
        [res.results[c]["y"].reshape(B, 2 * T_FULL, 2 * H, 2 * W)[:, 1:]
         for c in range(8)], axis=1)
    return out.astype(np.float32)


# revision 9
# speedup vs baseline: 1.0193x; 1.0193x over previous
"""Inverse 3D Haar wavelet transform (stride-2 kernel-2 conv_transpose) on 8 trn2 cores.

coeffs: [4, 64, 17, 128, 128] f32, channel dim = 8 subbands x 8 channels.
out:    [4, 8, 33, 256, 256] f32,
  out[b,c,2t+i-1, 2h+j, 2w+k] = 0.3536 * sum_s (-1)^(i*s2 + j*s1 + k*s0) x[b,s,c,t,h,w]
  (frame t'=-1 dropped).

Sharding: pure data parallel over the 8 channels c (one per core); each core
sees its [4, 8, 17, 128, 128] slice and emits [4, 33, 256, 256].

This kernel runs fp16 end-to-end on device (graded tolerance is 2e-2; fp16
butterflies land ~4e-4), halving HBM traffic vs f32: 17.8 MB in + 17.8 MB out
per core ~= 98 us at the 360 GB/s DMA roofline.

Host side: pre-scale by 0.3536, cast fp16, transpose to [bt=68, h=128, (s,w)]
so loads are 2 KB-contiguous per (bt, h). Output is [136, 256, 256] with frame
f = 2*bt + i; each b's first frame (t=0, i=0, the dropped t'=-1) lands on
f = 34*b which the host discards - keeps every device store uniform.

Device per 4-bt chunk (17 chunks, partition dim = h = 128):
  - one 1 MB load DMA (SP queue)
  - DVE: stage1 (s2->i) 2 ops, stage2 (s1->j) 2 ops, stage3 k=0 add - all
    packed fp16 = 2x DVE mode
  - GPSIMD: stage3 k=1 sub
  - ACT: interleave copy (t,i,j,k,w) -> (t,i,j,w'=2w+k)
  - one 1 MB store DMA (ACT queue)
"""

import sys

sys.path.insert(0, "/opt/trn_rl_repo")

import numpy as np

import concourse.bass as bass
import concourse.bacc as bacc
import concourse.mybir as mybir
from concourse.tile import TileContext
from concourse import bass_utils

B, S, C, T_FULL, H, W = 4, 8, 8, 17, 128, 128
SCALE = 0.3536
NBT = B * T_FULL  # 68 flattened (b, t) slices
BT_CHUNK = 4      # bt slices per inner iteration (68 = 4 * 17, no runts)

_cache = {}


def _build():
    f16 = mybir.dt.float16
    nc = bacc.Bacc()
    x = nc.dram_tensor("x", [NBT, H, S * W], f16, kind="ExternalInput")
    y = nc.dram_tensor("y", [2 * NBT, 2 * H, 2 * W], f16, kind="ExternalOutput")

    # small chunks at the ends shorten pipeline fill and drain
    chunks = [1, 1, 2] + [4] * 15 + [2, 1, 1]
    assert sum(chunks) == NBT
    # frames 34*b (b's t'=-1, i.e. i=0 of bt = 17*b) are dropped by the host;
    # skip them in the stores
    garbage_f = {2 * 17 * b for b in range(B)}
    with TileContext(nc) as tc:
        with tc.tile_pool(name="xin", bufs=4) as xp, \
             tc.tile_pool(name="u", bufs=3) as up, \
             tc.tile_pool(name="v", bufs=3) as vp, \
             tc.tile_pool(name="o", bufs=3) as op_, \
             tc.tile_pool(name="f", bufs=4) as fp:
            bt0 = 0
            for cidx, TC in enumerate(chunks):
                tail = cidx >= len(chunks) - 3
                FD = TC * S * W
                # ---- load: one DMA, 2 KB descriptors per (bt, h)
                X = xp.tile([H, FD], f16, tag="x")
                nc.sync.dma_start(
                    out=X[:].rearrange("p (t f) -> p t f", t=TC),
                    in_=x[bt0:bt0 + TC].rearrange("t p f -> p t f"))
                # ---- stage 1 (DVE): u[i] = x[s2=0] +/- x[s2=1]
                U = up.tile([H, FD], f16, tag="u")
                X3 = X[:].rearrange("p (t h) -> p t h", t=TC)          # h=(s,w) 1024
                U3 = U[:].rearrange("p (t i h) -> p t i h", t=TC, i=2)  # h=(s1 s0 w) 512
                nc.vector.tensor_add(U3[:, :, 0], X3[:, :, :512], X3[:, :, 512:])
                nc.vector.tensor_sub(U3[:, :, 1], X3[:, :, :512], X3[:, :, 512:])
                # ---- stage 2 (DVE): v[i,j] = u[i,s1=0] +/- u[i,s1=1]
                V = vp.tile([H, FD], f16, tag="v")
                U4 = U[:].rearrange("p (t i s1 g) -> p t i s1 g", t=TC, i=2, s1=2)
                V4 = V[:].rearrange("p (t i j g) -> p t i j g", t=TC, i=2, j=2)
                nc.vector.tensor_add(V4[:, :, :, 0], U4[:, :, :, 0], U4[:, :, :, 1])
                nc.vector.tensor_sub(V4[:, :, :, 1], U4[:, :, :, 0], U4[:, :, :, 1])
                # ---- stage 3: o[i,j,k] = v[i,j,s0=0] +/- v[i,j,s0=1]
                #      k=1 on GPSIMD; k=0 split DVE/GPSIMD to balance the two
                #      (packed writes keep DVE in its 2x fp16 mode)
                O = op_.tile([H, FD], f16, tag="o")
                V5 = V[:].rearrange("p (t ij s0 w) -> p t ij s0 w", t=TC, ij=4, s0=2)
                O5 = O[:].rearrange("p (t ij k w) -> p t ij k w", t=TC, ij=4, k=2)
                nc.gpsimd.tensor_sub(O5[:, :, :, 1], V5[:, :, :, 0], V5[:, :, :, 1])
                tsp = TC - 1 if TC == 4 else TC  # last t of big chunks -> GPSIMD
                nc.vector.tensor_add(O5[:, :tsp, :, 0],
                                     V5[:, :tsp, :, 0], V5[:, :tsp, :, 1])
                if tsp < TC:
                    nc.gpsimd.tensor_add(O5[:, tsp:, :, 0],
                                         V5[:, tsp:, :, 0], V5[:, tsp:, :, 1])
                # ---- interleave copy (ACT): (t,i,j,k,w) -> (t,i,j, w'=2w+k)
                #      (per-frame on the tail chunks so stores can drain early)
                F = fp.tile([H, FD], f16, tag="f")
                Fv = F[:].rearrange("p (g j w k) -> p g j k w", g=2 * TC, j=2, k=2)
                Og = O[:].rearrange("p (g j k w) -> p g j k w", g=2 * TC, j=2, k=2)
                nc.scalar.copy(out=Fv, in_=Og)
                # ---- store frame ranges (skipping dropped frames);
                #      F free = (g, j, w') with g = 2*t_local + i
                f0 = 2 * bt0
                Fg = F[:].rearrange("p (g j w) -> p g j w", g=2 * TC, j=2)
                cuts = sorted(f - f0 for f in garbage_f if f0 <= f < f0 + 2 * TC)
                bounds = [0] + [c for cut in cuts
                                for c in (cut, cut + 1)] + [2 * TC]
                pairs = list(zip(bounds[::2], bounds[1::2]))
                for ga, gb in pairs:
                    if ga == gb:
                        continue
                    dst = y[f0 + ga:f0 + gb].rearrange("g (p j) w -> p g j w", j=2)
                    nc.scalar.dma_start(out=dst, in_=Fg[:, ga:gb])
                bt0 += TC
    nc.finalize()
    return nc


def kernel(coeffs: np.ndarray) -> np.ndarray:
    coeffs = np.asarray(coeffs, dtype=np.float32)
    if "nc" not in _cache:
        _cache["nc"] = _build()
    nc = _cache["nc"]
    in_maps = []
    for c in range(8):
        xc = coeffs[:, c::8]  # [b, s, t, h, w] subband-major channel slice
        xt = (xc * np.float32(SCALE)).astype(np.float16).transpose(0, 2, 3, 1, 4)
        in_maps.append({"x": np.ascontiguousarray(xt).reshape(NBT, H, S * W)})
    res = bass_utils.run_bass_kernel_spmd(nc, in_maps, core_ids=list(range(8)))
    # y frame f = 2*bt + i = 34*b + 2*t + i; frame 34*b is the dropped t'=-1
    out = np.stack(
        [res.results[c]["y"].reshape(B, 2 * T_FULL, 2 * H, 2 * W)[:, 1:]
         for c in range(8)], axis=1)
    return out.astype(np.float32)


# revision 12
# speedup vs baseline: 1.0275x; 1.0080x over previous
"""Inverse 3D Haar wavelet transform (stride-2 kernel-2 conv_transpose) on 8 trn2 cores.

coeffs: [4, 64, 17, 128, 128] f32, channel dim = 8 subbands x 8 channels.
out:    [4, 8, 33, 256, 256] f32,
  out[b,c,2t+i-1, 2h+j, 2w+k] = 0.3536 * sum_s (-1)^(i*s2 + j*s1 + k*s0) x[b,s,c,t,h,w]
  (frame t'=-1 dropped).

Sharding: pure data parallel over the 8 channels c (one per core); each core
sees its [4, 8, 17, 128, 128] slice and emits [4, 33, 256, 256].

This kernel runs fp16 end-to-end on device (graded tolerance is 2e-2; fp16
butterflies land ~4e-4), halving HBM traffic vs f32: 17.8 MB in + 17.8 MB out
per core ~= 98 us at the 360 GB/s DMA roofline.

Host side: pre-scale by 0.3536, cast fp16, transpose to [bt=68, h=128, (s,w)]
so loads are 2 KB-contiguous per (bt, h). Output is [136, 256, 256] with frame
f = 2*bt + i; each b's first frame (t=0, i=0, the dropped t'=-1) lands on
f = 34*b which the host discards - keeps every device store uniform.

Device per 4-bt chunk (17 chunks, partition dim = h = 128):
  - one 1 MB load DMA (SP queue)
  - DVE: stage1 (s2->i) 2 ops, stage2 (s1->j) 2 ops, stage3 k=0 add - all
    packed fp16 = 2x DVE mode
  - GPSIMD: stage3 k=1 sub
  - ACT: interleave copy (t,i,j,k,w) -> (t,i,j,w'=2w+k)
  - one 1 MB store DMA (ACT queue)
"""

import sys

sys.path.insert(0, "/opt/trn_rl_repo")

import numpy as np

import concourse.bass as bass
import concourse.bacc as bacc
import concourse.mybir as mybir
from concourse.tile import TileContext
from concourse import bass_utils

B, S, C, T_FULL, H, W = 4, 8, 8, 17, 128, 128
SCALE = 0.3536
NBT = B * T_FULL  # 68 flattened (b, t) slices
BT_CHUNK = 4      # bt slices per inner iteration (68 = 4 * 17, no runts)

_cache = {}


def _build():
    f16 = mybir.dt.float16
    nc = bacc.Bacc()
    x = nc.dram_tensor("x", [NBT, H, S * W], f16, kind="ExternalInput")
    y = nc.dram_tensor("y", [2 * NBT, 2 * H, 2 * W], f16, kind="ExternalOutput")

    # small chunks at the ends shorten pipeline fill and drain
    chunks = [1, 1, 2] + [4] * 15 + [2, 1, 1]
    assert sum(chunks) == NBT
    # frames 34*b (b's t'=-1, i.e. i=0 of bt = 17*b) are dropped by the host;
    # skip them in the stores
    garbage_f = {2 * 17 * b for b in range(B)}
    with TileContext(nc) as tc:
        with tc.tile_pool(name="xin", bufs=4) as xp, \
             tc.tile_pool(name="u", bufs=3) as up, \
             tc.tile_pool(name="v", bufs=3) as vp, \
             tc.tile_pool(name="o", bufs=3) as op_, \
             tc.tile_pool(name="f", bufs=4) as fp:
            bt0 = 0
            for cidx, TC in enumerate(chunks):
                tail = cidx >= len(chunks) - 3
                head = cidx < 3
                FD = TC * S * W
                # ---- load: one DMA, 2 KB descriptors per (bt, h)
                X = xp.tile([H, FD], f16, tag="x")
                nc.sync.dma_start(
                    out=X[:].rearrange("p (t f) -> p t f", t=TC),
                    in_=x[bt0:bt0 + TC].rearrange("t p f -> p t f"))
                # ---- stage 1 (DVE): u[i] = x[s2=0] +/- x[s2=1]
                U = up.tile([H, FD], f16, tag="u")
                X3 = X[:].rearrange("p (t h) -> p t h", t=TC)          # h=(s,w) 1024
                U3 = U[:].rearrange("p (t i h) -> p t i h", t=TC, i=2)  # h=(s1 s0 w) 512
                nc.vector.tensor_add(U3[:, :, 0], X3[:, :, :512], X3[:, :, 512:])
                nc.vector.tensor_sub(U3[:, :, 1], X3[:, :, :512], X3[:, :, 512:])
                # ---- stage 2 (DVE): v[i,j] = u[i,s1=0] +/- u[i,s1=1]
                V = vp.tile([H, FD], f16, tag="v")
                U4 = U[:].rearrange("p (t i s1 g) -> p t i s1 g", t=TC, i=2, s1=2)
                V4 = V[:].rearrange("p (t i j g) -> p t i j g", t=TC, i=2, j=2)
                nc.vector.tensor_add(V4[:, :, :, 0], U4[:, :, :, 0], U4[:, :, :, 1])
                nc.vector.tensor_sub(V4[:, :, :, 1], U4[:, :, :, 0], U4[:, :, :, 1])
                # ---- stage 3: o[i,j,k] = v[i,j,s0=0] +/- v[i,j,s0=1]
                #      k=1 on GPSIMD; k=0 split DVE/GPSIMD to balance the two
                #      (packed writes keep DVE in its 2x fp16 mode)
                O = op_.tile([H, FD], f16, tag="o")
                V5 = V[:].rearrange("p (t ij s0 w) -> p t ij s0 w", t=TC, ij=4, s0=2)
                O5 = O[:].rearrange("p (t ij k w) -> p t ij k w", t=TC, ij=4, k=2)
                nc.gpsimd.tensor_sub(O5[:, :, :, 1], V5[:, :, :, 0], V5[:, :, :, 1])
                # k=0: GPSIMD takes it all on head chunks (lets DVE run ahead),
                # the last t of big chunks mid-stream, nothing on tail chunks
                # (short post-DVE chain into the final stores)
                tsp = 0 if head else (TC - 1 if TC == 4 else TC)
                if tsp > 0:
                    nc.vector.tensor_add(O5[:, :tsp, :, 0],
                                         V5[:, :tsp, :, 0], V5[:, :tsp, :, 1])
                if tsp < TC:
                    nc.gpsimd.tensor_add(O5[:, tsp:, :, 0],
                                         V5[:, tsp:, :, 0], V5[:, tsp:, :, 1])
                # ---- interleave copy (ACT): (t,i,j,k,w) -> (t,i,j, w'=2w+k)
                #      (split by k on tail chunks so the k=0 half overlaps
                #       GPSIMD's k=1 butterfly)
                F = fp.tile([H, FD], f16, tag="f")
                Fv = F[:].rearrange("p (g j w k) -> p g j k w", g=2 * TC, j=2, k=2)
                Og = O[:].rearrange("p (g j k w) -> p g j k w", g=2 * TC, j=2, k=2)
                if tail:
                    nc.scalar.copy(out=Fv[:, :, :, 0], in_=Og[:, :, :, 0])
                    nc.scalar.copy(out=Fv[:, :, :, 1], in_=Og[:, :, :, 1])
                else:
                    nc.scalar.copy(out=Fv, in_=Og)
                # ---- store frame ranges (skipping dropped frames);
                #      F free = (g, j, w') with g = 2*t_local + i
                f0 = 2 * bt0
                Fg = F[:].rearrange("p (g j w) -> p g j w", g=2 * TC, j=2)
                cuts = sorted(f - f0 for f in garbage_f if f0 <= f < f0 + 2 * TC)
                bounds = [0] + [c for cut in cuts
                                for c in (cut, cut + 1)] + [2 * TC]
                pairs = list(zip(bounds[::2], bounds[1::2]))
                # tail stores ride the (by then idle) SP load queue
                store_eng = nc.sync if tail else nc.scalar
                for ga, gb in pairs:
                    if ga == gb:
                        continue
                    dst = y[f0 + ga:f0 + gb].rearrange("g (p j) w -> p g j w", j=2)
                    store_eng.dma_start(out=dst, in_=Fg[:, ga:gb])
                bt0 += TC
    nc.finalize()
    return nc


def kernel(coeffs: np.ndarray) -> np.ndarray:
    coeffs = np.asarray(coeffs, dtype=np.float32)
    if "nc" not in _cache:
        _cache["nc"] = _build()
    nc = _cache["nc"]
    in_maps = []
    for c in range(8):
        xc = coeffs[:, c::8]  # [b, s, t, h, w] subband-major channel slice
        xt = (xc * np.float32(SCALE)).astype(np.float16).transpose(0, 2, 3, 1, 4)
        in_maps.append({"x": np.ascontiguousarray(xt).reshape(NBT, H, S * W)})
    res = bass_utils.run_bass_kernel_spmd(nc, in_maps, core_ids=list(range(8)))
    # y frame f = 2*bt + i = 34*b + 2*t + i; frame 34*b is the dropped t'=-1
    out = np.stack(
        [res.results[c]["y"].reshape(B, 2 * T_FULL, 2 * H, 2 * W)[:, 1:]
         for c in range(8)], axis=1)
    return out.astype(np.float32)


# revision 21
# speedup vs baseline: 1.0539x; 1.0257x over previous
"""Inverse 3D Haar wavelet transform (stride-2 kernel-2 conv_transpose) on 8 trn2 cores.

coeffs: [4, 64, 17, 128, 128] f32, channel dim = 8 subbands x 8 channels.
out:    [4, 8, 33, 256, 256] f32,
  out[b,c,2t+i-1, 2h+j, 2w+k] = 0.3536 * sum_s (-1)^(i*s2 + j*s1 + k*s0) x[b,s,c,t,h,w]
  (frame t'=-1 dropped).

Sharding: pure data parallel over the 8 channels c (one per core); each core
sees its [4, 8, 17, 128, 128] slice and emits [4, 33, 256, 256].

This kernel runs fp16 end-to-end on device (graded tolerance is 2e-2; fp16
butterflies land ~4e-4), halving HBM traffic vs f32: 17.8 MB in + 17.8 MB out
per core ~= 98 us at the 360 GB/s DMA roofline.

Host side: pre-scale by 0.3536, cast fp16, transpose to [bt=68, h=128, (s,w)]
so loads are 2 KB-contiguous per (bt, h). Output is [136, 256, 256] with frame
f = 2*bt + i; each b's first frame (t=0, i=0, the dropped t'=-1) lands on
f = 34*b which the host discards - keeps every device store uniform.

Device per 4-bt chunk (17 chunks, partition dim = h = 128):
  - one 1 MB load DMA (SP queue)
  - DVE: stage1 (s2->i) 2 ops, stage2 (s1->j) 2 ops, stage3 k=0 add - all
    packed fp16 = 2x DVE mode
  - GPSIMD: stage3 k=1 sub
  - ACT: interleave copy (t,i,j,k,w) -> (t,i,j,w'=2w+k)
  - one 1 MB store DMA (ACT queue)
"""

import sys

sys.path.insert(0, "/opt/trn_rl_repo")

import numpy as np

import concourse.bass as bass
import concourse.bacc as bacc
import concourse.mybir as mybir
from concourse.tile import TileContext
from concourse import bass_utils

B, S, C, T_FULL, H, W = 4, 8, 8, 17, 128, 128
SCALE = 0.3536
NBT = B * T_FULL  # 68 flattened (b, t) slices
CHUNKS = [1, 1, 2] + [4] * 14 + [2, 2] + [1] * 4  # bt slices per inner iteration
BUFS = (6, 4, 4, 4, 5)  # xin, u, v, o, f tile-pool depths
TAILCOPY = "ksplit_mix"  # tail-chunk interleave-copy engine strategy
NTAIL = 5               # how many trailing chunks get tail treatment
TAILK = "pool"          # tail butterfly engine: dve | pool

_cache = {}


def _build():
    f16 = mybir.dt.float16
    nc = bacc.Bacc()
    x = nc.dram_tensor("x", [NBT, H, S * W], f16, kind="ExternalInput")
    y = nc.dram_tensor("y", [2 * NBT, 2 * H, 2 * W], f16, kind="ExternalOutput")

    # small chunks at the ends shorten pipeline fill and drain
    chunks = CHUNKS
    assert sum(chunks) == NBT
    # frames 34*b (b's t'=-1, i.e. i=0 of bt = 17*b) are dropped by the host;
    # skip them in the stores
    garbage_f = {2 * 17 * b for b in range(B)}
    with TileContext(nc) as tc:
        with tc.tile_pool(name="xin", bufs=BUFS[0]) as xp, \
             tc.tile_pool(name="u", bufs=BUFS[1]) as up, \
             tc.tile_pool(name="v", bufs=BUFS[2]) as vp, \
             tc.tile_pool(name="o", bufs=BUFS[3]) as op_, \
             tc.tile_pool(name="f", bufs=BUFS[4]) as fp:
            bt0 = 0
            for cidx, TC in enumerate(chunks):
                tail = cidx >= len(chunks) - NTAIL
                head = cidx < 3
                FD = TC * S * W
                # ---- load: one DMA, 2 KB descriptors per (bt, h)
                X = xp.tile([H, FD], f16, tag="x")
                nc.sync.dma_start(
                    out=X[:].rearrange("p (t f) -> p t f", t=TC),
                    in_=x[bt0:bt0 + TC].rearrange("t p f -> p t f"))
                # ---- stage 1 (DVE): u[i] = x[s2=0] +/- x[s2=1]
                U = up.tile([H, FD], f16, tag="u")
                X3 = X[:].rearrange("p (t h) -> p t h", t=TC)          # h=(s,w) 1024
                U3 = U[:].rearrange("p (t i h) -> p t i h", t=TC, i=2)  # h=(s1 s0 w) 512
                nc.vector.tensor_add(U3[:, :, 0], X3[:, :, :512], X3[:, :, 512:])
                nc.vector.tensor_sub(U3[:, :, 1], X3[:, :, :512], X3[:, :, 512:])
                # ---- stage 2 (DVE): v[i,j] = u[i,s1=0] +/- u[i,s1=1]
                V = vp.tile([H, FD], f16, tag="v")
                U4 = U[:].rearrange("p (t i s1 g) -> p t i s1 g", t=TC, i=2, s1=2)
                V4 = V[:].rearrange("p (t i j g) -> p t i j g", t=TC, i=2, j=2)
                nc.vector.tensor_add(V4[:, :, :, 0], U4[:, :, :, 0], U4[:, :, :, 1])
                nc.vector.tensor_sub(V4[:, :, :, 1], U4[:, :, :, 0], U4[:, :, :, 1])
                # ---- stage 3: o[i,j,k] = v[i,j,s0=0] +/- v[i,j,s0=1]
                #      k=1 on GPSIMD; k=0 split DVE/GPSIMD to balance the two
                #      (packed writes keep DVE in its 2x fp16 mode)
                O = op_.tile([H, FD], f16, tag="o")
                V5 = V[:].rearrange("p (t ij s0 w) -> p t ij s0 w", t=TC, ij=4, s0=2)
                O5 = O[:].rearrange("p (t ij k w) -> p t ij k w", t=TC, ij=4, k=2)
                # k=1: GPSIMD mid-stream, DVE on tail chunks (everything ends
                # on the fast engine so the final copy/store chain starts early)
                (nc.vector if tail and TAILK == "dve" else nc.gpsimd).tensor_sub(
                    O5[:, :, :, 1], V5[:, :, :, 0], V5[:, :, :, 1])
                # k=0: all on GPSIMD for head chunks (lets DVE run ahead),
                # last t of big chunks on GPSIMD, all on DVE for tail chunks
                tsp = 0 if head else (TC - 1 if TC == 4 else TC)
                if tsp > 0:
                    nc.vector.tensor_add(O5[:, :tsp, :, 0],
                                         V5[:, :tsp, :, 0], V5[:, :tsp, :, 1])
                if tsp < TC:
                    nc.gpsimd.tensor_add(O5[:, tsp:, :, 0],
                                         V5[:, tsp:, :, 0], V5[:, tsp:, :, 1])
                # ---- interleave copy: (t,i,j,k,w) -> (t,i,j, w'=2w+k)
                #      ACT mid-stream; tail chunks per TAILCOPY config
                F = fp.tile([H, FD], f16, tag="f")
                Fv = F[:].rearrange("p (g j w k) -> p g j k w", g=2 * TC, j=2, k=2)
                Og = O[:].rearrange("p (g j k w) -> p g j k w", g=2 * TC, j=2, k=2)
                if tail and TAILCOPY == "dve":
                    nc.vector.tensor_copy(out=Fv, in_=Og)
                elif tail and TAILCOPY == "ksplit_act":
                    nc.scalar.copy(out=Fv[:, :, :, 0], in_=Og[:, :, :, 0])
                    nc.scalar.copy(out=Fv[:, :, :, 1], in_=Og[:, :, :, 1])
                elif tail and TAILCOPY == "ksplit_mix":
                    nc.scalar.copy(out=Fv[:, :, :, 0], in_=Og[:, :, :, 0])
                    nc.vector.tensor_copy(out=Fv[:, :, :, 1], in_=Og[:, :, :, 1])
                else:
                    nc.scalar.copy(out=Fv, in_=Og)
                # ---- store frame ranges (skipping dropped frames);
                #      F free = (g, j, w') with g = 2*t_local + i
                f0 = 2 * bt0
                Fg = F[:].rearrange("p (g j w) -> p g j w", g=2 * TC, j=2)
                cuts = sorted(f - f0 for f in garbage_f if f0 <= f < f0 + 2 * TC)
                bounds = [0] + [c for cut in cuts
                                for c in (cut, cut + 1)] + [2 * TC]
                pairs = list(zip(bounds[::2], bounds[1::2]))
                # tail stores ride the (by then idle) SP load queue
                store_eng = nc.sync if tail else nc.scalar
                for ga, gb in pairs:
                    if ga == gb:
                        continue
                    dst = y[f0 + ga:f0 + gb].rearrange("g (p j) w -> p g j w", j=2)
                    store_eng.dma_start(out=dst, in_=Fg[:, ga:gb])
                bt0 += TC
    nc.finalize()
    return nc


def kernel(coeffs: np.ndarray) -> np.ndarray:
    coeffs = np.asarray(coeffs, dtype=np.float32)
    if "nc" not in _cache:
        _cache["nc"] = _build()
    nc = _cache["nc"]
    in_maps = []
    for c in range(8):
        xc = coeffs[:, c::8]  # [b, s, t, h, w] subband-major channel slice
        xt = (xc * np.float32(SCALE)).astype(np.float16).transpose(0, 2, 3, 1, 4)
        in_maps.append({"x": np.ascontiguousarray(xt).reshape(NBT, H, S * W)})
    res = bass_utils.run_bass_kernel_spmd(nc, in_maps, core_ids=list(range(8)))
    # y frame f = 2*bt + i = 34*b + 2*t + i; frame 34*b is the dropped t'=-1
    out = np.stack(
        [res.results[c]["y"].reshape(B, 2 * T_FULL, 2 * H, 2 * W)[:, 1:]
         for c in range(8)], axis=1)
    return out.astype(np.float32)


# revision 22
# speedup vs baseline: 1.0618x; 1.0075x over previous
"""Inverse 3D Haar wavelet transform (stride-2 kernel-2 conv_transpose) on 8 trn2 cores.

coeffs: [4, 64, 17, 128, 128] f32, channel dim = 8 subbands x 8 channels.
out:    [4, 8, 33, 256, 256] f32,
  out[b,c,2t+i-1, 2h+j, 2w+k] = 0.3536 * sum_s (-1)^(i*s2 + j*s1 + k*s0) x[b,s,c,t,h,w]
  (frame t'=-1 dropped).

Sharding: pure data parallel over the 8 channels c (one per core); each core
sees its [4, 8, 17, 128, 128] slice and emits [4, 33, 256, 256].

This kernel runs fp16 end-to-end on device (graded tolerance is 2e-2; fp16
butterflies land ~4e-4), halving HBM traffic vs f32: 17.8 MB in + 17.8 MB out
per core ~= 98 us at the 360 GB/s DMA roofline.

Host side: pre-scale by 0.3536, cast fp16, transpose to [bt=68, h=128, (s,w)]
so loads are 2 KB-contiguous per (bt, h). Output is [136, 256, 256] with frame
f = 2*bt + i; each b's first frame (t=0, i=0, the dropped t'=-1) lands on
f = 34*b which the host discards - keeps every device store uniform.

Device per 4-bt chunk (17 chunks, partition dim = h = 128):
  - one 1 MB load DMA (SP queue)
  - DVE: stage1 (s2->i) 2 ops, stage2 (s1->j) 2 ops, stage3 k=0 add - all
    packed fp16 = 2x DVE mode
  - GPSIMD: stage3 k=1 sub
  - ACT: interleave copy (t,i,j,k,w) -> (t,i,j,w'=2w+k)
  - one 1 MB store DMA (ACT queue)
"""

import sys

sys.path.insert(0, "/opt/trn_rl_repo")

import numpy as np

import concourse.bass as bass
import concourse.bacc as bacc
import concourse.mybir as mybir
from concourse.tile import TileContext
from concourse import bass_utils

B, S, C, T_FULL, H, W = 4, 8, 8, 17, 128, 128
SCALE = 0.3536
NBT = B * T_FULL  # 68 flattened (b, t) slices
CHUNKS = [1, 1, 2] + [4] * 13 + [2] * 4 + [1] * 4  # bt slices per inner iteration
BUFS = (6, 4, 4, 4, 5)  # xin, u, v, o, f tile-pool depths
TAILCOPY = "ksplit_mix"  # tail-chunk interleave-copy engine strategy
NTAIL = 5               # how many trailing chunks get tail treatment
TAILK = "pool"          # tail butterfly engine: dve | pool

_cache = {}


def _build():
    f16 = mybir.dt.float16
    nc = bacc.Bacc()
    x = nc.dram_tensor("x", [NBT, H, S * W], f16, kind="ExternalInput")
    y = nc.dram_tensor("y", [2 * NBT, 2 * H, 2 * W], f16, kind="ExternalOutput")

    # small chunks at the ends shorten pipeline fill and drain
    chunks = CHUNKS
    assert sum(chunks) == NBT
    # frames 34*b (b's t'=-1, i.e. i=0 of bt = 17*b) are dropped by the host;
    # skip them in the stores
    garbage_f = {2 * 17 * b for b in range(B)}
    with TileContext(nc) as tc:
        with tc.tile_pool(name="xin", bufs=BUFS[0]) as xp, \
             tc.tile_pool(name="u", bufs=BUFS[1]) as up, \
             tc.tile_pool(name="v", bufs=BUFS[2]) as vp, \
             tc.tile_pool(name="o", bufs=BUFS[3]) as op_, \
             tc.tile_pool(name="f", bufs=BUFS[4]) as fp:
            bt0 = 0
            for cidx, TC in enumerate(chunks):
                tail = cidx >= len(chunks) - NTAIL
                head = cidx < 3
                FD = TC * S * W
                # ---- load: one DMA, 2 KB descriptors per (bt, h)
                X = xp.tile([H, FD], f16, tag="x")
                nc.sync.dma_start(
                    out=X[:].rearrange("p (t f) -> p t f", t=TC),
                    in_=x[bt0:bt0 + TC].rearrange("t p f -> p t f"))
                # ---- stage 1 (DVE): u[i] = x[s2=0] +/- x[s2=1]
                U = up.tile([H, FD], f16, tag="u")
                X3 = X[:].rearrange("p (t h) -> p t h", t=TC)          # h=(s,w) 1024
                U3 = U[:].rearrange("p (t i h) -> p t i h", t=TC, i=2)  # h=(s1 s0 w) 512
                nc.vector.tensor_add(U3[:, :, 0], X3[:, :, :512], X3[:, :, 512:])
                nc.vector.tensor_sub(U3[:, :, 1], X3[:, :, :512], X3[:, :, 512:])
                # ---- stage 2 (DVE): v[i,j] = u[i,s1=0] +/- u[i,s1=1]
                V = vp.tile([H, FD], f16, tag="v")
                U4 = U[:].rearrange("p (t i s1 g) -> p t i s1 g", t=TC, i=2, s1=2)
                V4 = V[:].rearrange("p (t i j g) -> p t i j g", t=TC, i=2, j=2)
                nc.vector.tensor_add(V4[:, :, :, 0], U4[:, :, :, 0], U4[:, :, :, 1])
                nc.vector.tensor_sub(V4[:, :, :, 1], U4[:, :, :, 0], U4[:, :, :, 1])
                # ---- stage 3: o[i,j,k] = v[i,j,s0=0] +/- v[i,j,s0=1]
                #      k=1 on GPSIMD; k=0 split DVE/GPSIMD to balance the two
                #      (packed writes keep DVE in its 2x fp16 mode)
                O = op_.tile([H, FD], f16, tag="o")
                V5 = V[:].rearrange("p (t ij s0 w) -> p t ij s0 w", t=TC, ij=4, s0=2)
                O5 = O[:].rearrange("p (t ij k w) -> p t ij k w", t=TC, ij=4, k=2)
                # k=1: GPSIMD mid-stream, DVE on tail chunks (everything ends
                # on the fast engine so the final copy/store chain starts early)
                (nc.vector if tail and TAILK == "dve" else nc.gpsimd).tensor_sub(
                    O5[:, :, :, 1], V5[:, :, :, 0], V5[:, :, :, 1])
                # k=0: all on GPSIMD for head chunks (lets DVE run ahead),
                # last t of big chunks on GPSIMD, all on DVE for tail chunks
                tsp = 0 if head else (TC - 1 if TC == 4 else TC)
                if tsp > 0:
                    nc.vector.tensor_add(O5[:, :tsp, :, 0],
                                         V5[:, :tsp, :, 0], V5[:, :tsp, :, 1])
                if tsp < TC:
                    nc.gpsimd.tensor_add(O5[:, tsp:, :, 0],
                                         V5[:, tsp:, :, 0], V5[:, tsp:, :, 1])
                # ---- interleave copy: (t,i,j,k,w) -> (t,i,j, w'=2w+k)
                #      ACT mid-stream; tail chunks per TAILCOPY config
                F = fp.tile([H, FD], f16, tag="f")
                Fv = F[:].rearrange("p (g j w k) -> p g j k w", g=2 * TC, j=2, k=2)
                Og = O[:].rearrange("p (g j k w) -> p g j k w", g=2 * TC, j=2, k=2)
                if tail and TAILCOPY == "dve":
                    nc.vector.tensor_copy(out=Fv, in_=Og)
                elif tail and TAILCOPY == "ksplit_act":
                    nc.scalar.copy(out=Fv[:, :, :, 0], in_=Og[:, :, :, 0])
                    nc.scalar.copy(out=Fv[:, :, :, 1], in_=Og[:, :, :, 1])
                elif tail and TAILCOPY == "ksplit_mix":
                    nc.scalar.copy(out=Fv[:, :, :, 0], in_=Og[:, :, :, 0])
                    nc.vector.tensor_copy(out=Fv[:, :, :, 1], in_=Og[:, :, :, 1])
                else:
                    nc.scalar.copy(out=Fv, in_=Og)
                # ---- store frame ranges (skipping dropped frames);
                #      F free = (g, j, w') with g = 2*t_local + i
                f0 = 2 * bt0
                Fg = F[:].rearrange("p (g j w) -> p g j w", g=2 * TC, j=2)
                cuts = sorted(f - f0 for f in garbage_f if f0 <= f < f0 + 2 * TC)
                bounds = [0] + [c for cut in cuts
                                for c in (cut, cut + 1)] + [2 * TC]
                pairs = list(zip(bounds[::2], bounds[1::2]))
                # tail stores ride the (by then idle) SP load queue
                store_eng = nc.sync if tail else nc.scalar
                for ga, gb in pairs:
                    if ga == gb:
                        continue
                    dst = y[f0 + ga:f0 + gb].rearrange("g (p j) w -> p g j w", j=2)
                    store_eng.dma_start(out=dst, in_=Fg[:, ga:gb])
                bt0 += TC
    nc.finalize()
    return nc


def kernel(coeffs: np.ndarray) -> np.ndarray:
    coeffs = np.asarray(coeffs, dtype=np.float32)
    if "nc" not in _cache:
        _cache["nc"] = _build()
    nc = _cache["nc"]
    in_maps = []
    for c in range(8):
        xc = coeffs[:, c::8]  # [b, s, t, h, w] subband-major channel slice
        xt = (xc * np.float32(SCALE)).astype(np.float16).transpose(0, 2, 3, 1, 4)
        in_maps.append({"x": np.ascontiguousarray(xt).reshape(NBT, H, S * W)})
    res = bass_utils.run_bass_kernel_spmd(nc, in_maps, core_ids=list(range(8)))
    # y frame f = 2*bt + i = 34*b + 2*t + i; frame 34*b is the dropped t'=-1
    out = np.stack(
        [res.results[c]["y"].reshape(B, 2 * T_FULL, 2 * H, 2 * W)[:, 1:]
         for c in range(8)], axis=1)
    return out.astype(np.float32)


# revision 26
# speedup vs baseline: 1.0870x; 1.0237x over previous
"""Inverse 3D Haar wavelet transform (stride-2 kernel-2 conv_transpose) on 8 trn2 cores.

coeffs: [4, 64, 17, 128, 128] f32, channel dim = 8 subbands x 8 channels.
out:    [4, 8, 33, 256, 256] f32,
  out[b,c,2t+i-1, 2h+j, 2w+k] = 0.3536 * sum_s (-1)^(i*s2 + j*s1 + k*s0) x[b,s,c,t,h,w]
  (frame t'=-1 dropped).

Sharding: pure data parallel over the 8 channels c (one per core); each core
sees its [4, 8, 17, 128, 128] slice and emits [4, 33, 256, 256].

This kernel runs fp16 end-to-end on device (graded tolerance is 2e-2; fp16
butterflies land ~4e-4), halving HBM traffic vs f32: 17.8 MB in + 17.8 MB out
per core ~= 98 us at the 360 GB/s DMA roofline.

Host side: pre-scale by 0.3536, cast fp16, transpose to [bt=68, h=128, (s,w)]
so loads are 2 KB-contiguous per (bt, h). Output is [136, 256, 256] with frame
f = 2*bt + i; each b's first frame (t=0, i=0, the dropped t'=-1) lands on
f = 34*b which the host discards - keeps every device store uniform.

Device per bt-chunk (mostly 4-bt chunks; smaller at the ends to shorten
pipeline fill/drain; partition dim = h = 128):
  - one load DMA (SP queue, 2 KB descriptors)
  - DVE: stage1 (s2->i), stage2 (s1->j), part of stage3 - all packed fp16
    TensorTensor = 2x DVE mode
  - GPSIMD: the rest of stage3 (balanced so DVE/Pool both stay under the
    DMA pace of ~5.8 us per 4-bt chunk)
  - interleave copy (t,i,j,k,w) -> (t,i,j,w'=2w+k): ACT mid-stream,
    DVE/ACT k-split on tail chunks so the final stores drain early
  - store DMA with 512 B descriptors (ACT queue; SP for tail chunks)
Cost-model floor is ~97.6 us of DMA; this lands ~104.4 us.
"""

import sys

sys.path.insert(0, "/opt/trn_rl_repo")

import numpy as np

import concourse.bass as bass
import concourse.bacc as bacc
import concourse.mybir as mybir
from concourse.tile import TileContext
from concourse import bass_utils

B, S, C, T_FULL, H, W = 4, 8, 8, 17, 128, 128
SCALE = 0.3536
NBT = B * T_FULL  # 68 flattened (b, t) slices
CHUNKS = [1, 1, 2] + [4] * 13 + [2] * 4 + [1] * 4  # bt slices per inner iteration
BUFS = (6, 4, 4, 4, 5)  # xin, u, v, o, f tile-pool depths
TAILCOPY = "act"         # tail-chunk interleave-copy engine strategy
NTAIL = 4               # how many trailing chunks get tail treatment
TAILK = "pool"          # tail butterfly engine: dve | pool

_cache = {}


def _build():
    f16 = mybir.dt.float16
    nc = bacc.Bacc()
    x = nc.dram_tensor("x", [NBT, H, S * W], f16, kind="ExternalInput")
    y = nc.dram_tensor("y", [2 * NBT, 2 * H, 2 * W], f16, kind="ExternalOutput")

    # small chunks at the ends shorten pipeline fill and drain
    chunks = CHUNKS
    assert sum(chunks) == NBT
    # frames 34*b (b's t'=-1, i.e. i=0 of bt = 17*b) are dropped by the host;
    # skip them in the stores
    garbage_f = {2 * 17 * b for b in range(B)}
    with TileContext(nc) as tc:
        with tc.tile_pool(name="xin", bufs=BUFS[0]) as xp, \
             tc.tile_pool(name="u", bufs=BUFS[1]) as up, \
             tc.tile_pool(name="v", bufs=BUFS[2]) as vp, \
             tc.tile_pool(name="o", bufs=BUFS[3]) as op_, \
             tc.tile_pool(name="f", bufs=BUFS[4]) as fp:
            bt0 = 0
            for cidx, TC in enumerate(chunks):
                tail = cidx >= len(chunks) - NTAIL
                head = cidx < 3
                FD = TC * S * W
                # ---- load: one DMA, 2 KB descriptors per (bt, h)
                X = xp.tile([H, FD], f16, tag="x")
                nc.sync.dma_start(
                    out=X[:].rearrange("p (t f) -> p t f", t=TC),
                    in_=x[bt0:bt0 + TC].rearrange("t p f -> p t f"))
                # ---- stage 1 (DVE): u[i] = x[s2=0] +/- x[s2=1]
                U = up.tile([H, FD], f16, tag="u")
                X3 = X[:].rearrange("p (t h) -> p t h", t=TC)          # h=(s,w) 1024
                U3 = U[:].rearrange("p (t i h) -> p t i h", t=TC, i=2)  # h=(s1 s0 w) 512
                nc.vector.tensor_add(U3[:, :, 0], X3[:, :, :512], X3[:, :, 512:])
                nc.vector.tensor_sub(U3[:, :, 1], X3[:, :, :512], X3[:, :, 512:])
                # ---- stage 2 (DVE): v[i,j] = u[i,s1=0] +/- u[i,s1=1]
                V = vp.tile([H, FD], f16, tag="v")
                U4 = U[:].rearrange("p (t i s1 g) -> p t i s1 g", t=TC, i=2, s1=2)
                V4 = V[:].rearrange("p (t i j g) -> p t i j g", t=TC, i=2, j=2)
                nc.vector.tensor_add(V4[:, :, :, 0], U4[:, :, :, 0], U4[:, :, :, 1])
                nc.vector.tensor_sub(V4[:, :, :, 1], U4[:, :, :, 0], U4[:, :, :, 1])
                # ---- stage 3: o[i,j,k] = v[i,j,s0=0] +/- v[i,j,s0=1]
                #      GPSIMD's share writes DIRECTLY into the w'-interleaved
                #      store layout F (its cost is stride-independent), which
                #      removes that share from the copy pass entirely. DVE's
                #      share stays packed (keeps its 2x fp16 mode) and is
                #      copied into F afterwards.
                V5 = V[:].rearrange("p (t ij s0 w) -> p t ij s0 w", t=TC, ij=4, s0=2)
                F = fp.tile([H, FD], f16, tag="f")
                Fd = F[:].rearrange("p (t ij w k) -> p t ij k w", t=TC, ij=4, k=2)
                # k=1 direct into F: GPSIMD mid-stream, DVE (1x, strided) on
                # tail chunks so everything ends on the fast engine
                (nc.vector if tail and TAILK == "dve" else nc.gpsimd).tensor_sub(
                    Fd[:, :, :, 1], V5[:, :, :, 0], V5[:, :, :, 1])
                # k=0: all on GPSIMD for head chunks (lets DVE run ahead),
                # last t of big chunks on GPSIMD, all on DVE for tail chunks
                tsp = 0 if head else (TC - 1 if TC == 4 else TC)
                if tsp > 0:
                    O = op_.tile([H, tsp * 512], f16, tag="o")
                    O5 = O[:].rearrange("p (t ij w) -> p t ij w", t=tsp, ij=4)
                    nc.vector.tensor_add(O5[:, :],
                                         V5[:, :tsp, :, 0], V5[:, :tsp, :, 1])
                    # interleave copy of DVE's packed k=0 share
                    ceng = nc.vector.tensor_copy if tail and TAILCOPY != "act" \
                        else nc.scalar.copy
                    ceng(out=Fd[:, :tsp, :, 0], in_=O5[:, :])
                if tsp < TC:
                    nc.gpsimd.tensor_add(Fd[:, tsp:, :, 0],
                                         V5[:, tsp:, :, 0], V5[:, tsp:, :, 1])
                # ---- store frame ranges (skipping dropped frames);
                #      F free = (g, j, w') with g = 2*t_local + i
                f0 = 2 * bt0
                Fg = F[:].rearrange("p (g j w) -> p g j w", g=2 * TC, j=2)
                cuts = sorted(f - f0 for f in garbage_f if f0 <= f < f0 + 2 * TC)
                bounds = [0] + [c for cut in cuts
                                for c in (cut, cut + 1)] + [2 * TC]
                pairs = list(zip(bounds[::2], bounds[1::2]))
                # tail stores ride the (by then idle) SP load queue
                store_eng = nc.sync if tail else nc.scalar
                for ga, gb in pairs:
                    if ga == gb:
                        continue
                    dst = y[f0 + ga:f0 + gb].rearrange("g (p j) w -> p g j w", j=2)
                    store_eng.dma_start(out=dst, in_=Fg[:, ga:gb])
                bt0 += TC
    nc.finalize()
    return nc


def kernel(coeffs: np.ndarray) -> np.ndarray:
    coeffs = np.asarray(coeffs, dtype=np.float32)
    if "nc" not in _cache:
        _cache["nc"] = _build()
    nc = _cache["nc"]
    in_maps = []
    for c in range(8):
        xc = coeffs[:, c::8]  # [b, s, t, h, w] subband-major channel slice
        xt = (xc * np.float32(SCALE)).astype(np.float16).transpose(0, 2, 3, 1, 4)
        in_maps.append({"x": np.ascontiguousarray(xt).reshape(NBT, H, S * W)})
    res = bass_utils.run_bass_kernel_spmd(nc, in_maps, core_ids=list(range(8)))
    # y frame f = 2*bt + i = 34*b + 2*t + i; frame 34*b is the dropped t'=-1
    out = np.stack(
        [res.results[c]["y"].reshape(B, 2 * T_FULL, 2 * H, 2 * W)[:, 1:]
         for c in range(8)], axis=1)
    return out.astype(np.float32)
